# revision 6
# baseline (speedup 1.0000x reference)
"""Trainium2 Bass kernel for capsule-network AgreementRouting (n_iterations=1).

Reference computation (see problem):
    c = softmax(b, axis=-1)                  # [IN, OUT] (same for every batch)
    s[b,o,d] = sum_in c[in,o] * u[b,in,o,d]  # weighted reduce over input caps
    v = squash(s)                            # per (b,o): s * l2/(1+l2)/sqrt(l2)
    out = v[:, None]                         # [B, 1, OUT, DIM]

Strategy: data-parallel over batch across 8 NeuronCores (64 batches/core).
Per core the 47 MB u-shard is streamed through SBUF with large contiguous
DMAs; the in-caps reduction is done on TensorE with softmax(b) chunks as the
stationary operand (out[o', (g,o,d)] = sum_in c[in,o'] u[in,g,o,d]), and the
o'==o diagonal is extracted with a precomputed identity mask + strided
reduce on VectorE. Squash runs on ACT/DVE over the tiny [10, 64, 16] result.
"""

import numpy as np

import concourse.bass as bass
import concourse.tile as tile
from concourse import bacc, mybir
from concourse.bass_utils import run_bass_kernel_spmd

F32 = mybir.dt.float32

B, IN_CAPS, OUT_CAPS, OUT_DIM = 512, 1152, 10, 16
N_CORES = 8
B_LOCAL = B // N_CORES            # 64 batches per core
OD = OUT_CAPS * OUT_DIM           # 160
P = 128                           # partitions
N_CHUNKS = IN_CAPS // P           # 9 contraction chunks
GROUP = 2                         # batches per PSUM accumulation group (320 cols)
TILE_B = 4                        # batches per DMA tile (~2.95 MB per DMA)
N_TILES = B_LOCAL // TILE_B       # 16
GROUPS_PER_TILE = TILE_B // GROUP # 2


def _build_core_program() -> bass.Bass:
    nc = bacc.Bacc(None)
    u = nc.dram_tensor("u", [B_LOCAL, IN_CAPS, OUT_CAPS, OUT_DIM], F32,
                       kind="ExternalInput")
    bp = nc.dram_tensor("b", [IN_CAPS, OUT_CAPS], F32, kind="ExternalInput")
    v = nc.dram_tensor("v", [OUT_CAPS, B_LOCAL, OUT_DIM], F32,
                       kind="ExternalOutput")

    # in-cap index mapping: in = p*N_CHUNKS + n (partition-major). Per (p, b)
    # the 9 chunk rows are contiguous in HBM -> 5760B runs per partition for
    # u and a single 360B run for b, keeping each DMA on one descriptor lane.
    u_r = u[:].rearrange("b (p n) o d -> p b n (o d)", p=P)
    b_r = bp[:].rearrange("(p n) o -> p n o", p=P)

    with tile.TileContext(nc) as tc:
        with (
            tc.tile_pool(name="singles", bufs=1) as singles,
            tc.tile_pool(name="inp", bufs=3) as inp,
            tc.tile_pool(name="psum", bufs=4, space="PSUM") as psum,
            tc.tile_pool(name="mids", bufs=4) as mids,
        ):
            # ---- softmax over b rows: c[in, o] ----
            b_sb = singles.tile([P, N_CHUNKS, OUT_CAPS], F32)
            nc.sync.dma_start(out=b_sb, in_=b_r)
            bmax = singles.tile([P, N_CHUNKS], F32)
            nc.vector.reduce_max(out=bmax, in_=b_sb, axis=mybir.AxisListType.X)
            negmax = singles.tile([P, N_CHUNKS], F32)
            nc.scalar.mul(out=negmax, in_=bmax, mul=-1.0)
            e_sb = singles.tile([P, N_CHUNKS, OUT_CAPS], F32)
            for n in range(N_CHUNKS):
                nc.scalar.activation(
                    out=e_sb[:, n, :], in_=b_sb[:, n, :],
                    func=mybir.ActivationFunctionType.Exp,
                    bias=negmax[:, n : n + 1], scale=1.0,
                )
            esum = singles.tile([P, N_CHUNKS], F32)
            nc.vector.reduce_sum(out=esum, in_=e_sb, axis=mybir.AxisListType.X)
            einv = singles.tile([P, N_CHUNKS], F32)
            nc.vector.reciprocal(out=einv, in_=esum)
            c_sb = singles.tile([P, N_CHUNKS, OUT_CAPS], F32)
            for n in range(N_CHUNKS):
                nc.vector.tensor_scalar_mul(
                    out=c_sb[:, n, :], in0=e_sb[:, n, :],
                    scalar1=einv[:, n : n + 1],
                )

            # ---- diagonal-selection mask: mask[o', g, o, d] = (o == o') ----
            mask = singles.tile([OUT_CAPS, GROUP, OUT_CAPS, OUT_DIM], F32)
            nc.gpsimd.memset(mask, 0.0)
            nc.gpsimd.affine_select(
                out=mask, in_=mask,
                compare_op=mybir.AluOpType.not_equal,
                fill=1.0, base=0, channel_multiplier=1,
                pattern=[[0, GROUP], [-1, OUT_CAPS], [0, OUT_DIM]],
            )

            # s[o, b, d] accumulated across all groups
            s_sb = singles.tile([OUT_CAPS, B_LOCAL, OUT_DIM], F32)

            # ---- main streaming loop ----
            for t in range(N_TILES):
                u_tile = inp.tile([P, TILE_B, N_CHUNKS, OD], F32)
                nc.sync.dma_start(
                    out=u_tile, in_=u_r[:, t * TILE_B : (t + 1) * TILE_B],
                )
                for g in range(GROUPS_PER_TILE):
                    ps = psum.tile([OUT_CAPS, GROUP, OD], F32)
                    for n in range(N_CHUNKS):
                        nc.tensor.matmul(
                            ps,
                            c_sb[:, n, :],
                            u_tile[:, g * GROUP : (g + 1) * GROUP, n, :],
                            start=(n == 0), stop=(n == N_CHUNKS - 1),
                        )
                    # mask off-diagonal (o' != o) then reduce over o
                    masked = mids.tile([OUT_CAPS, GROUP, OUT_CAPS, OUT_DIM], F32)
                    nc.vector.tensor_mul(
                        out=masked,
                        in0=ps.rearrange("q g (o d) -> q g o d", d=OUT_DIM),
                        in1=mask,
                    )
                    b0 = (t * GROUPS_PER_TILE + g) * GROUP
                    nc.vector.reduce_sum(
                        out=s_sb[:, b0 : b0 + GROUP, :],
                        in_=masked.rearrange("q g o d -> q g d o"),
                        axis=mybir.AxisListType.X,
                    )

            # ---- squash: v = s * l2/(1+l2)/sqrt(l2) along d ----
            sq = singles.tile([OUT_CAPS, B_LOCAL, OUT_DIM], F32)
            nc.vector.tensor_mul(out=sq, in0=s_sb, in1=s_sb)
            l2 = singles.tile([OUT_CAPS, B_LOCAL], F32)
            nc.vector.reduce_sum(out=l2, in_=sq, axis=mybir.AxisListType.X)
            rt = singles.tile([OUT_CAPS, B_LOCAL], F32)
            nc.scalar.sqrt(out=rt, in_=l2)
            denom = singles.tile([OUT_CAPS, B_LOCAL], F32)
            nc.scalar.add(out=denom, in_=l2, add=1.0)
            dinv = singles.tile([OUT_CAPS, B_LOCAL], F32)
            nc.vector.reciprocal(out=dinv, in_=denom)
            scl = singles.tile([OUT_CAPS, B_LOCAL], F32)
            nc.vector.tensor_mul(out=scl, in0=rt, in1=dinv)
            v_sb = singles.tile([OUT_CAPS, B_LOCAL, OUT_DIM], F32)
            for d in range(OUT_DIM):
                nc.vector.tensor_mul(
                    out=v_sb[:, :, d], in0=s_sb[:, :, d], in1=scl,
                )
            nc.sync.dma_start(out=v[:], in_=v_sb)

    nc.compile()
    return nc


_NC_CACHE = None


def _get_program() -> bass.Bass:
    global _NC_CACHE
    if _NC_CACHE is None:
        _NC_CACHE = _build_core_program()
    return _NC_CACHE


def kernel(u_predict: np.ndarray, b: np.ndarray, n_iterations) -> np.ndarray:
    u_predict = np.ascontiguousarray(np.asarray(u_predict, dtype=np.float32))
    b = np.ascontiguousarray(np.asarray(b, dtype=np.float32))
    nc = _get_program()
    in_maps = [
        {"u": u_predict[i * B_LOCAL : (i + 1) * B_LOCAL], "b": b}
        for i in range(N_CORES)
    ]
    results = run_bass_kernel_spmd(nc, in_maps, list(range(N_CORES))).results
    # per-core v is [OUT_CAPS, B_LOCAL, OUT_DIM] -> assemble [B, OUT, DIM]
    vs = np.stack([results[i]["v"] for i in range(N_CORES)])
    out = vs.transpose(0, 2, 1, 3).reshape(B, OUT_CAPS, OUT_DIM)
    if int(n_iterations) >= 1:
        out = out[:, None]
    return np.ascontiguousarray(out.astype(np.float32))


# revision 8
# speedup vs baseline: 1.4047x; 1.4047x over previous
"""Trainium2 Bass kernel for capsule-network AgreementRouting (n_iterations=1).

Reference computation (see problem):
    c = softmax(b, axis=-1)                  # [IN, OUT] (same for every batch)
    s[b,o,d] = sum_in c[in,o] * u[b,in,o,d]  # weighted reduce over input caps
    v = squash(s)                            # per (b,o): s * l2/(1+l2)/sqrt(l2)
    out = v[:, None]                         # [B, 1, OUT, DIM]

Strategy: data-parallel over batch across 8 NeuronCores (64 batches/core).
Per core the 47 MB u-shard is streamed through SBUF with large contiguous
DMAs; the in-caps reduction is done on TensorE with softmax(b) chunks as the
stationary operand (out[o', (g,o,d)] = sum_in c[in,o'] u[in,g,o,d]), and the
o'==o diagonal is extracted with a precomputed identity mask + strided
reduce on VectorE. Squash runs on ACT/DVE over the tiny [10, 64, 16] result.
"""

import numpy as np

import concourse.bass as bass
import concourse.tile as tile
from concourse import bacc, mybir
from concourse.bass_utils import run_bass_kernel_spmd

F32 = mybir.dt.float32
F32R = mybir.dt.float32r

B, IN_CAPS, OUT_CAPS, OUT_DIM = 512, 1152, 10, 16
N_CORES = 8
B_LOCAL = B // N_CORES            # 64 batches per core
OD = OUT_CAPS * OUT_DIM           # 160
P = 128                           # partitions
N_CHUNKS = IN_CAPS // P           # 9 contraction chunks
GROUP = 2                         # batches per PSUM accumulation group (320 cols)
TILE_B = 4                        # batches per DMA tile (~2.95 MB per DMA)
N_TILES = B_LOCAL // TILE_B       # 16
GROUPS_PER_TILE = TILE_B // GROUP # 2


def _build_core_program() -> bass.Bass:
    nc = bacc.Bacc(None)
    u = nc.dram_tensor("u", [B_LOCAL, IN_CAPS, OUT_CAPS, OUT_DIM], F32,
                       kind="ExternalInput")
    bp = nc.dram_tensor("b", [IN_CAPS, OUT_CAPS], F32, kind="ExternalInput")
    v = nc.dram_tensor("v", [OUT_CAPS, B_LOCAL, OUT_DIM], F32,
                       kind="ExternalOutput")

    # in-cap index mapping: in = p*N_CHUNKS + n (partition-major). Per (p, b)
    # the 9 chunk rows are contiguous in HBM -> 5760B runs per partition for
    # u and a single 360B run for b, keeping each DMA on one descriptor lane.
    u_r = u[:].rearrange("b (p n) o d -> p b n (o d)", p=P)
    b_r = bp[:].rearrange("(p n) o -> p n o", p=P)

    with tile.TileContext(nc) as tc:
        with (
            tc.tile_pool(name="singles", bufs=1) as singles,
            tc.tile_pool(name="inp", bufs=3) as inp,
            tc.tile_pool(name="psum", bufs=4, space="PSUM") as psum,
            tc.tile_pool(name="mids", bufs=4) as mids,
        ):
            # ---- softmax over b rows: c[in, o] ----
            b_sb = singles.tile([P, N_CHUNKS, OUT_CAPS], F32)
            nc.sync.dma_start(out=b_sb, in_=b_r)
            bmax = singles.tile([P, N_CHUNKS], F32)
            nc.vector.reduce_max(out=bmax, in_=b_sb, axis=mybir.AxisListType.X)
            negmax = singles.tile([P, N_CHUNKS], F32)
            nc.scalar.mul(out=negmax, in_=bmax, mul=-1.0)
            e_sb = singles.tile([P, N_CHUNKS, OUT_CAPS], F32)
            for n in range(N_CHUNKS):
                nc.scalar.activation(
                    out=e_sb[:, n, :], in_=b_sb[:, n, :],
                    func=mybir.ActivationFunctionType.Exp,
                    bias=negmax[:, n : n + 1], scale=1.0,
                )
            esum = singles.tile([P, N_CHUNKS], F32)
            nc.vector.reduce_sum(out=esum, in_=e_sb, axis=mybir.AxisListType.X)
            einv = singles.tile([P, N_CHUNKS], F32)
            nc.vector.reciprocal(out=einv, in_=esum)
            c_sb = singles.tile([P, N_CHUNKS, OUT_CAPS], F32R)
            for n in range(N_CHUNKS):
                nc.vector.tensor_scalar_mul(
                    out=c_sb[:, n, :], in0=e_sb[:, n, :],
                    scalar1=einv[:, n : n + 1],
                )

            # ---- diagonal-selection mask: mask[o', g, o, d] = (o == o') ----
            mask = singles.tile([OUT_CAPS, GROUP, OUT_CAPS, OUT_DIM], F32)
            nc.gpsimd.memset(mask, 0.0)
            nc.gpsimd.affine_select(
                out=mask, in_=mask,
                compare_op=mybir.AluOpType.not_equal,
                fill=1.0, base=0, channel_multiplier=1,
                pattern=[[0, GROUP], [-1, OUT_CAPS], [0, OUT_DIM]],
            )

            # s[o, b, d] accumulated across all groups
            s_sb = singles.tile([OUT_CAPS, B_LOCAL, OUT_DIM], F32)

            # ---- main streaming loop ----
            for t in range(N_TILES):
                u_tile = inp.tile([P, TILE_B, N_CHUNKS, OD], F32R)
                nc.sync.dma_start(
                    out=u_tile,
                    in_=u_r[:, t * TILE_B : (t + 1) * TILE_B].bitcast(F32R),
                )
                for g in range(GROUPS_PER_TILE):
                    ps = psum.tile([OUT_CAPS, GROUP, OD], F32)
                    for n in range(N_CHUNKS):
                        # float32r: fp32 bits, single-pass (tf32-like) matmul
                        nc.tensor.matmul(
                            ps,
                            c_sb[:, n, :],
                            u_tile[:, g * GROUP : (g + 1) * GROUP, n, :],
                            start=(n == 0), stop=(n == N_CHUNKS - 1),
                        )
                    # mask off-diagonal (o' != o) then reduce over o
                    masked = mids.tile([OUT_CAPS, GROUP, OUT_CAPS, OUT_DIM], F32)
                    nc.vector.tensor_mul(
                        out=masked,
                        in0=ps.rearrange("q g (o d) -> q g o d", d=OUT_DIM),
                        in1=mask,
                    )
                    b0 = (t * GROUPS_PER_TILE + g) * GROUP
                    nc.vector.reduce_sum(
                        out=s_sb[:, b0 : b0 + GROUP, :],
                        in_=masked.rearrange("q g o d -> q g d o"),
                        axis=mybir.AxisListType.X,
                    )

            # ---- squash: v = s * l2/(1+l2)/sqrt(l2) along d ----
            sq = singles.tile([OUT_CAPS, B_LOCAL, OUT_DIM], F32)
            nc.vector.tensor_mul(out=sq, in0=s_sb, in1=s_sb)
            l2 = singles.tile([OUT_CAPS, B_LOCAL], F32)
            nc.vector.reduce_sum(out=l2, in_=sq, axis=mybir.AxisListType.X)
            rt = singles.tile([OUT_CAPS, B_LOCAL], F32)
            nc.scalar.sqrt(out=rt, in_=l2)
            denom = singles.tile([OUT_CAPS, B_LOCAL], F32)
            nc.scalar.add(out=denom, in_=l2, add=1.0)
            dinv = singles.tile([OUT_CAPS, B_LOCAL], F32)
            nc.vector.reciprocal(out=dinv, in_=denom)
            scl = singles.tile([OUT_CAPS, B_LOCAL], F32)
            nc.vector.tensor_mul(out=scl, in0=rt, in1=dinv)
            v_sb = singles.tile([OUT_CAPS, B_LOCAL, OUT_DIM], F32)
            for d in range(OUT_DIM):
                nc.vector.tensor_mul(
                    out=v_sb[:, :, d], in0=s_sb[:, :, d], in1=scl,
                )
            nc.sync.dma_start(out=v[:], in_=v_sb)

    nc.compile()
    return nc


_NC_CACHE = None


def _get_program() -> bass.Bass:
    global _NC_CACHE
    if _NC_CACHE is None:
        _NC_CACHE = _build_core_program()
    return _NC_CACHE


def kernel(u_predict: np.ndarray, b: np.ndarray, n_iterations) -> np.ndarray:
    u_predict = np.ascontiguousarray(np.asarray(u_predict, dtype=np.float32))
    b = np.ascontiguousarray(np.asarray(b, dtype=np.float32))
    nc = _get_program()
    in_maps = [
        {"u": u_predict[i * B_LOCAL : (i + 1) * B_LOCAL], "b": b}
        for i in range(N_CORES)
    ]
    results = run_bass_kernel_spmd(nc, in_maps, list(range(N_CORES))).results
    # per-core v is [OUT_CAPS, B_LOCAL, OUT_DIM] -> assemble [B, OUT, DIM]
    vs = np.stack([results[i]["v"] for i in range(N_CORES)])
    out = vs.transpose(0, 2, 1, 3).reshape(B, OUT_CAPS, OUT_DIM)
    if int(n_iterations) >= 1:
        out = out[:, None]
    return np.ascontiguousarray(out.astype(np.float32))
